# revision 43
# baseline (speedup 1.0000x reference)
"""Trainium2 Bass kernel for nn_MultiHeadDuelingDQN (8-core SPMD), bf16 edition.

Model (B=256, STATE=26240, H=512, R=4000, N=64 heads, M=10):
    h  = relu(relu(x@W1+b1)@W2+b2)
    q_cache = h@Wvc+bvc + (h@Wac+bac) - mean_R(h@Wac+bac)
    q_assoc = per-head dueling over M (local means)
    q_rec   = S - mean_R(S),  S = h @ (sum_n Wru[n]) + sum_n bru[n]
              [exact rewrite: rec_global has zero row-mean, so the
              reference's second mean subtraction folds away]

Sharding (8 cores), all streamed tensors staged as bf16 on host:
  - fc1: contraction (STATE) split 8 ways. x and W1 are pre-transposed /
    supertiled on the host so fc1 is pure matmul (no on-device transposes).
    Partial h1 [256,512] summed across cores via ReduceScatter (bf16, CCE
    adds in the DMA path); each core computes fc2 on its 32 batch rows and
    AllGathers h2 (bf16).
  - rec/cache: R split 8 ways (500 cols/core). sum_n Wru[n] becomes DVE
    grouped reduces over big [128, 16000] bf16 chunks (heads innermost),
    then one small matmul h @ W_sum per batch tile. Row-means over the full
    R use a tiny [128,4] f32 AllGather + local reduce.
  - assoc heads: 8 heads/core, fully local; augmented matmul
    [Wau | Wvu | Wvc] -> [adv_assoc | val_n | value_c] in one pass.
Engine split: PE does fc/head matmuls + the few h transposes; DVE only the
Wru stream reduction; ACT does PSUM->SBUF copies/relus/casts, row-sums
(accum_out), mean subtraction (Identity+bias) and load-DMA dispatch; Sync
dispatches the big input streams (xT, W1, Wru); GpSimd runs collectives,
output stores and small SBUF elementwise ops.

kernel(**inputs) takes full unsharded f32 inputs, returns full [256, 8640] f32.
"""
import numpy as np
import ml_dtypes

import concourse.bass as bass
import concourse.mybir as mybir
import concourse.tile as tile
from concourse import bacc
from concourse import bass_utils
from concourse.bass import ts
from concourse.masks import make_identity

NC = 8
B, H, STATE, R, NH, M = 256, 512, 26240, 4000, 64, 10
KPC_RAW = STATE // NC          # 3280
KCH = 26                       # k-chunks of 128 per core (padded)
KPC = KCH * 128                # 3328
RPC = R // NC                  # 500
HPC = NH // NC                 # 8 heads per core
AUG = HPC * (M + 1) + 1        # 89 = [8x(10 adv + 1 val)] + value_c
BPC = B // NC                  # 32 batch rows per core for fc2
F32 = mybir.dt.float32
BF16 = mybir.dt.bfloat16
BF = ml_dtypes.bfloat16
RELU = mybir.ActivationFunctionType.Relu
COPY = mybir.ActivationFunctionType.Copy
IDENT = mybir.ActivationFunctionType.Identity
ADD = mybir.AluOpType.add

# wru stream chunking: (kc, r-offset, r-count) pieces; last chunk halved so
# the final DVE tree (which gates S) is short
RN = 125
WRU_CHUNKS = ([(kc, rq * RN, RN) for kc in range(4) for rq in range(4)][:-1]
              + [(3, 375, 62), (3, 437, 63)])


def build_program(wru_bufs=4):
    nc = bacc.Bacc("TRN2", target_bir_lowering=False, debug=False, num_devices=NC)

    # ---- per-core I/O (host pre-packs layouts; see make_in_maps) ----
    xt = nc.dram_tensor("xt", [128, KCH * 256], BF16, kind="ExternalInput").ap()
    w1g = [nc.dram_tensor(f"w1g{g}", [128, 13 * 512], BF16, kind="ExternalInput").ap()
           for g in range(2)]
    b1 = nc.dram_tensor("b1", [1, H], BF16, kind="ExternalInput").ap()
    w2 = nc.dram_tensor("w2", [128, 4 * 512], BF16, kind="ExternalInput").ap()
    b2 = nc.dram_tensor("b2", [1, H], BF16, kind="ExternalInput").ap()
    wac = nc.dram_tensor("wac", [128, 4 * RPC], BF16, kind="ExternalInput").ap()
    bac = nc.dram_tensor("bac", [1, RPC], BF16, kind="ExternalInput").ap()
    # [kc, p, r*64 + n] = Wru[n, kc*128+p, r0+r]  (heads innermost)
    wru = nc.dram_tensor("wru", [4, 128, RPC * NH], BF16, kind="ExternalInput").ap()
    bru = nc.dram_tensor("bru", [NH, RPC], BF16, kind="ExternalInput").ap()
    aug = nc.dram_tensor("aug", [128, 4 * AUG], BF16, kind="ExternalInput").ap()
    augb = nc.dram_tensor("augb", [1, AUG], BF16, kind="ExternalInput").ap()
    # selection matrix: sel4[g*32+p, p] = 1 — sums 4 stacked partials via PE
    sel4 = nc.dram_tensor("sel4", [128, BPC], BF16, kind="ExternalInput").ap()

    out_cache = nc.dram_tensor("out_cache", [B, RPC], F32, kind="ExternalOutput").ap()
    out_rec = nc.dram_tensor("out_rec", [B, RPC], F32, kind="ExternalOutput").ap()
    out_assoc = nc.dram_tensor("out_assoc", [B, HPC * M], F32, kind="ExternalOutput").ap()

    with tile.TileContext(nc) as tc:
        with (
            tc.tile_pool(name="cst", bufs=1) as cst,
            tc.tile_pool(name="sb", bufs=1) as sb,
            tc.tile_pool(name="wrup", bufs=wru_bufs) as wrup,
            tc.tile_pool(name="redp", bufs=2) as redp,
            tc.tile_pool(name="ps", bufs=2, space="PSUM") as ps,
            tc.tile_pool(name="psfc", bufs=2, space="PSUM") as psfc,
            tc.tile_pool(name="dram", bufs=1, space="DRAM") as dram,
        ):
            ident = cst.tile([128, 128], BF16, tag="ident")
            make_identity(nc, ident)
            ones8 = cst.tile([1, 128], BF16, tag="ones8")
            nc.vector.memset(ones8, 1.0 / NC)
            ones1 = cst.tile([1, 128], BF16, tag="ones1")
            nc.vector.memset(ones1, 1.0)
            ones64 = cst.tile([64, 128], BF16, tag="ones64")
            nc.vector.memset(ones64, 1.0)

            # ---------- Sync HWDGE ring: trunk inputs first, then wru stream --
            xt_sb = sb.tile([128, KCH * 256], BF16, tag="xt_sb")
            nc.sync.dma_start(xt_sb, xt)
            w1_sb = []
            for g in range(2):
                t = sb.tile([128, 13 * 512], BF16, tag=f"w1_{g}", name=f"w1_{g}")
                nc.sync.dma_start(t, w1g[g])
                w1_sb.append(t)

            # ---------- small loads on the Scalar HWDGE ring ----------
            b1row = cst.tile([1, H], BF16, tag="b1row")
            nc.scalar.dma_start(b1row, b1)
            w2_sb = sb.tile([128, 4 * 512], BF16, tag="w2_sb")
            nc.scalar.dma_start(w2_sb, w2)
            b2row = cst.tile([1, H], BF16, tag="b2row")
            nc.scalar.dma_start(b2row, b2)
            wac_sb = sb.tile([128, 4 * RPC], BF16, tag="wac_sb")
            nc.scalar.dma_start(wac_sb, wac)
            bac_row = cst.tile([1, RPC], BF16, tag="bac_row")
            nc.scalar.dma_start(bac_row, bac)
            aug_sb = cst.tile([128, 4 * AUG], BF16, tag="aug_sb")
            nc.scalar.dma_start(aug_sb, aug)
            augb_row = cst.tile([1, AUG], BF16, tag="augb_row")
            nc.scalar.dma_start(augb_row, augb)
            bru_sb = sb.tile([64, RPC], BF16, tag="bru_sb")
            nc.scalar.dma_start(bru_sb, bru)
            a_col = [sb.tile([128, 1], BF16, tag=f"acol{k}", name=f"acol{k}")
                     for k in range(4)]
            brs_f = sb.tile([64, 1], F32, tag="brs_f")
            brs = sb.tile([64, 1], BF16, tag="brs")
            junk64 = sb.tile([64, RPC], F32, tag="junk64")
            nc.scalar.activation(junk64, bru_sb, COPY, accum_out=brs_f)
            nc.scalar.copy(brs, brs_f)

            # wru stream + DVE head-sum as a binary TT tree (bf16 2x packed
            # mode; tensor_reduce only runs 1x on TRN2)
            acc_bf = [sb.tile([128, RPC], BF16, tag=f"accb{k}", name=f"accb{k}")
                      for k in range(4)]
            # ---------- Phase A: fc1 (pure matmul; x/W1 pre-transposed) ----
            h1_ps = [psfc.tile([128, H], F32, tag="fc", name=f"h1_ps{bt}")
                     for bt in range(2)]
            for bt in range(2):  # fold b1/8 first, opens the accumulation group
                nc.tensor.matmul(h1_ps[bt], ones8, b1row, start=True, stop=False)
            for kc in range(KCH):
                w1t = w1_sb[kc // 13]
                j = kc % 13
                for bt in range(2):
                    nc.tensor.matmul(h1_ps[bt],
                                     xt_sb[:, kc * 256 + bt * 128:kc * 256 + (bt + 1) * 128],
                                     w1t[:, ts(j, 512)],
                                     start=False, stop=(kc == KCH - 1))
            h1c = []
            rs_in_early = dram.tile([B, H], BF16, tag="rs_in")
            for bt in range(2):
                t = sb.tile([128, H], BF16, tag=f"h1c{bt}", name=f"h1c{bt}")
                nc.scalar.copy(t, h1_ps[bt])
                nc.gpsimd.dma_start(rs_in_early[ts(bt, 128), :], t)
                h1c.append(t)

            with nc.allow_low_precision(reason="DVE rounds each tree level to bf16"):
                for ci, (kc, rb, rn) in enumerate(WRU_CHUNKS):
                    wt = wrup.tile([128, rn * NH], BF16, tag="wru", name=f"wru_t{ci}")
                    nc.sync.dma_start(wt, wru[kc, :, rb * NH:(rb + rn) * NH])
                    sA = redp.tile([128, rn * 32], BF16, tag="sA", name=f"sA{ci}")
                    sB = redp.tile([128, rn * 16], BF16, tag="sB", name=f"sB{ci}")
                    cur = wt
                    dsts = [sA[:, 0:rn * 32], sB[:, 0:rn * 16], sA[:, 0:rn * 8],
                            sB[:, 0:rn * 4], sA[:, 0:rn * 2],
                            acc_bf[kc][:, rb:rb + rn]]
                    for lvl, g in enumerate([64, 32, 16, 8, 4, 2]):
                        h = g // 2
                        in0 = bass.AP(cur.tensor, cur.offset,
                                      [cur.ap[0], [g, rn], [1, h]])
                        in1 = bass.AP(cur.tensor, cur.offset + h,
                                      [cur.ap[0], [g, rn], [1, h]])
                        nc.vector.tensor_tensor(out=dsts[lvl], in0=in0, in1=in1,
                                                op=ADD)
                        cur = dsts[lvl]
                    if rb + rn == RPC:  # kc complete: a[k] = sum_r acc[k, :]
                        nc.vector.tensor_reduce(a_col[kc], acc_bf[kc],
                                                axis=mybir.AxisListType.X,
                                                op=ADD)

            # AllToAll h1 (bf16): one-hop mesh exchange; rank c receives the 8
            # partials of its 32 batch rows, then sums them with a PE sel-matmul
            rs_in = rs_in_early
            rs_out = dram.tile([B, H], BF16, tag="rs_out")
            nc.gpsimd.collective_compute(
                "AllToAll", mybir.AluOpType.bypass,
                replica_groups=[list(range(NC))],
                ins=[rs_in.opt()], outs=[rs_out.opt()],
            )
            # contiguous readback of the 8 stacked partials, then sum them
            # across partitions with two PE matmuls against sel4
            sel_sb = cst.tile([128, BPC], BF16, tag="sel_sb")
            nc.scalar.dma_start(sel_sb, sel4)
            h1sum_ps = psfc.tile([BPC, H], F32, tag="fc", name="h1sum_ps")
            for bt in range(2):
                h1pt = sb.tile([128, H], BF16, tag=f"h1pt{bt}", name=f"h1pt{bt}")
                nc.gpsimd.dma_start(h1pt, rs_out[ts(bt, 128), :])
                nc.tensor.matmul(h1sum_ps, sel_sb, h1pt,
                                 start=(bt == 0), stop=(bt == 1))
            h1s = sb.tile([BPC, H], BF16, tag="h1s")
            nc.scalar.activation(h1s, h1sum_ps, RELU)
            h1cT = []
            for kc in range(4):
                pt = ps.tile([128, BPC], BF16, tag="small", bufs=2, name=f"pth{kc}")
                nc.tensor.transpose(pt, h1s[:, ts(kc, 128)], ident[0:BPC, 0:BPC])
                t = sb.tile([128, BPC], BF16, tag=f"h1cT{kc}", name=f"h1cT{kc}")
                nc.scalar.copy(t, pt)
                h1cT.append(t)
            h2_ps = psfc.tile([BPC, H], F32, tag="fc", name="h2_ps")
            nc.tensor.matmul(h2_ps, ones1[:, 0:BPC], b2row, start=True, stop=False)
            for kc in range(4):
                nc.tensor.matmul(h2_ps, h1cT[kc], w2_sb[:, ts(kc, 512)],
                                 start=False, stop=(kc == 3))
            h2s = sb.tile([BPC, H], BF16, tag="h2s")
            nc.scalar.activation(h2s, h2_ps, RELU)
            ag_in = dram.tile([BPC, H], BF16, tag="ag_in")
            ag_out = dram.tile([B, H], BF16, tag="ag_out")
            nc.gpsimd.dma_start(ag_in, h2s)
            nc.gpsimd.collective_compute(
                "AllGather", mybir.AluOpType.bypass,
                replica_groups=[list(range(NC))],
                ins=[ag_in.opt()], outs=[ag_out.opt()],
            )
            # h2 [256, 512] -> hT chunks [128(h), 256(b)]
            hT = [sb.tile([128, B], BF16, tag=f"hT{kc}", name=f"hT{kc}")
                  for kc in range(4)]
            for bt in range(2):
                h2g = sb.tile([128, H], BF16, tag=f"h2g{bt}", name=f"h2g{bt}")
                nc.gpsimd.dma_start(h2g, ag_out[ts(bt, 128), :])
                for kc in range(4):
                    pt = ps.tile([128, 128], BF16, tag="small", bufs=2,
                                 name=f"ptg{bt}_{kc}")
                    nc.tensor.transpose(pt, h2g[:, ts(kc, 128)], ident)
                    nc.scalar.copy(hT[kc][:, ts(bt, 128)], pt)

            # S row-sums via h.a + sum(bru): available as soon as hT lands,
            # ~6us before the S matmuls + copies would deliver them
            ar2_in = sb.tile([128, 4], F32, tag="ar2_in")
            for bt in range(2):
                rs_ps = psfc.tile([128, 1], F32, tag="fc", name=f"rs_ps{bt}")
                for kc in range(4):
                    nc.tensor.matmul(rs_ps, hT[kc][:, ts(bt, 128)], a_col[kc],
                                     start=(kc == 0), stop=False)
                nc.tensor.matmul(rs_ps, ones64, brs, start=False, stop=True)
                nc.scalar.copy(ar2_in[:, 2 + bt:3 + bt], rs_ps)

            # ---------- Phase B: assoc heads (augmented [adv|val|value_c]) ----
            # (matmul+copy only; the dueling finalize is emitted after the
            # ar2-critical psC/psS copies so it stays off that ACT path)
            value_sb, psA_sbs = [], []
            for bt in range(2):
                psA = ps.tile([128, AUG], F32, tag="wide", bufs=3, name=f"psA{bt}")
                nc.tensor.matmul(psA, ones1, augb_row, start=True, stop=False)
                for kc in range(4):
                    nc.tensor.matmul(psA, hT[kc][:, ts(bt, 128)],
                                     aug_sb[:, kc * AUG:(kc + 1) * AUG],
                                     start=False, stop=(kc == 3))
                psA_sb = sb.tile([128, AUG], F32, tag=f"psAsb{bt}", name=f"psAsb{bt}")
                nc.scalar.copy(psA_sb, psA)
                psA_sbs.append(psA_sb)
                value_sb.append(psA_sb[:, AUG - 1:AUG])

            # ---------- Phase C: cache head (R-slice) ----------
            adv_c_sb = []
            for bt in range(2):
                psC = ps.tile([128, RPC], F32, tag="wide", bufs=3, name=f"psC{bt}")
                nc.tensor.matmul(psC, ones1, bac_row, start=True, stop=False)
                for kc in range(4):
                    nc.tensor.matmul(psC, hT[kc][:, ts(bt, 128)],
                                     wac_sb[:, kc * RPC:(kc + 1) * RPC],
                                     start=False, stop=(kc == 3))
                t = sb.tile([128, RPC], F32, tag=f"advc{bt}", name=f"advc{bt}")
                # copy + row-sum in one ACT pass (accum_out)
                nc.scalar.activation(t, psC, COPY,
                                     accum_out=ar2_in[:, bt:bt + 1])
                adv_c_sb.append(t)

            # ---------- Phase D: S = hT.T @ W_sum (+ sum_n bru fold) ----------
            s_sb = []
            for bt in range(2):
                t = ps.tile([128, RPC], F32, tag="wide", bufs=3, name=f"psS{bt}")
                nc.tensor.matmul(t, ones64, bru_sb, start=True, stop=False)
                for kc in range(4):
                    nc.tensor.matmul(t, hT[kc][:, ts(bt, 128)], acc_bf[kc],
                                     start=False, stop=(kc == 3))
                st = sb.tile([128, RPC], F32, tag=f"ssb{bt}", name=f"ssb{bt}")
                nc.scalar.activation(st, t, COPY)
                s_sb.append(st)

            # ---------- assoc dueling finalize (off the ar2 ACT path) ----------
            for bt in range(2):
                psA_sb = psA_sbs[bt]
                advs = psA_sb[:, 0:HPC * (M + 1)].rearrange("p (n u) -> p n u", u=M + 1)
                negm = sb.tile([128, HPC], F32, tag=f"negmA{bt}", name=f"negmA{bt}")
                junkA = sb.tile([128, M], F32, tag=f"junkA{bt}", name=f"junkA{bt}")
                for n in range(HPC):
                    nc.scalar.activation(junkA, advs[:, n, 0:M], COPY,
                                         scale=-1.0 / M,
                                         accum_out=negm[:, n:n + 1])
                tmp = sb.tile([128, HPC], F32, tag=f"tmpA{bt}", name=f"tmpA{bt}")
                nc.gpsimd.tensor_add(tmp, advs[:, :, M], negm)
                q = sb.tile([128, HPC * M], F32, tag=f"qA{bt}", name=f"qA{bt}")
                nc.gpsimd.tensor_tensor(
                    out=q.rearrange("p (n m) -> p n m", m=M),
                    in0=advs[:, :, 0:M],
                    in1=tmp.broadcast_to([128, HPC, M]),
                    op=ADD)
                nc.scalar.dma_start(out_assoc[ts(bt, 128), :], q)

            # ---------- Phase E: tiny AllGather of row-sums, finalize ----------
            ar2_din = dram.tile([128, 4], F32, tag="ar2_din")
            ar2_dout = dram.tile([NC * 128, 4], F32, tag="ar2_dout")
            nc.sync.dma_start(ar2_din, ar2_in)
            nc.gpsimd.collective_compute(
                "AllGather", mybir.AluOpType.bypass,
                replica_groups=[list(range(NC))],
                ins=[ar2_din.opt()], outs=[ar2_dout.opt()],
            )
            # one strided readback [128, (g,c)] then a single X-reduce over g
            rall = sb.tile([128, NC * 4], F32, tag="rall")
            nc.sync.dma_start(rall, ar2_dout.rearrange("(g p) c -> p g c", p=128))
            rview = bass.AP(rall.tensor, rall.offset,
                            [rall.ap[0], [1, 4], [4, NC]])
            ar2_sb = sb.tile([128, 4], F32, tag="ar2_sb")
            nc.vector.tensor_reduce(ar2_sb, rview, axis=mybir.AxisListType.X, op=ADD)
            negmeans = sb.tile([128, 4], F32, tag="negmeans")
            nc.scalar.activation(negmeans, ar2_sb, COPY, scale=-1.0 / R)

            for bt in range(2):
                vm = sb.tile([128, 1], F32, tag=f"vm{bt}", name=f"vm{bt}")
                nc.gpsimd.tensor_add(vm, value_sb[bt], negmeans[:, bt:bt + 1])
                qc = sb.tile([128, RPC], F32, tag=f"qc{bt}", name=f"qc{bt}")
                nc.scalar.activation(qc, adv_c_sb[bt], IDENT, bias=vm, scale=1.0)
                nc.scalar.dma_start(out_cache[ts(bt, 128), :], qc)

                qr = sb.tile([128, RPC], F32, tag=f"qr{bt}", name=f"qr{bt}")
                nc.scalar.activation(qr, s_sb[bt], IDENT,
                                     bias=negmeans[:, 2 + bt:3 + bt], scale=1.0)
                nc.scalar.dma_start(out_rec[ts(bt, 128), :], qr)

    nc.compile()
    return nc


_CACHED = None


def _get_program():
    global _CACHED
    if _CACHED is None:
        _CACHED = build_program()
    return _CACHED


def make_in_maps(x, W1, b1, W2, b2, Wvc, bvc, Wac, bac, Wvu, bvu, Wau, bau, Wru, bru):
    f = np.float32
    x = np.asarray(x, f)
    W1 = np.asarray(W1, f)
    W2 = np.asarray(W2, f)
    Wac = np.asarray(Wac, f)
    Wru = np.asarray(Wru, f)
    Wau = np.asarray(Wau, f)
    Wvu = np.asarray(Wvu, f)
    Wvc = np.asarray(Wvc, f).reshape(H)
    in_maps = []
    sel4p = np.zeros((128, BPC), np.float32)
    sel4p[np.arange(128), np.arange(128) % BPC] = 1.0
    sel4p = sel4p.astype(BF)
    w2p = np.ascontiguousarray(
        W2.reshape(4, 128, H).transpose(1, 0, 2)).reshape(128, 4 * H).astype(BF)
    b1r = np.asarray(b1, f).reshape(1, H).astype(BF)
    b2r = np.asarray(b2, f).reshape(1, H).astype(BF)
    for c in range(NC):
        k0 = c * KPC_RAW
        r0 = c * RPC
        h0 = c * HPC
        # xT supertile: [p, kc*256 + b] = x[b, k0 + kc*128 + p]
        xs = np.zeros((KPC, B), f)
        xs[:KPC_RAW] = x[:, k0:k0 + KPC_RAW].T
        xtp = xs.reshape(KCH, 128, B).transpose(1, 0, 2).reshape(128, KCH * B)
        # W1 supertiles: [p, j*512 + h] = W1[k0 + (g*13+j)*128 + p, h]
        w1s = np.zeros((KPC, H), f)
        w1s[:KPC_RAW] = W1[k0:k0 + KPC_RAW]
        w1r = w1s.reshape(KCH, 128, H)
        # wru chunks: [kc, p, r*64 + n] = Wru[n, kc*128+p, r0+r]
        ws = Wru[:, :, r0:r0 + RPC]                    # [64, 512, 500]
        wrup = np.ascontiguousarray(
            ws.reshape(NH, 4, 128, RPC).transpose(1, 2, 3, 0)).reshape(
            4, 128, RPC * NH)
        # wac supertile: [p, kc*500 + r] = Wac[kc*128+p, r0+r]
        wacp = np.ascontiguousarray(
            Wac[:, r0:r0 + RPC].reshape(4, 128, RPC).transpose(1, 0, 2)).reshape(
            128, 4 * RPC)
        # augmented assoc weights: [p, kc*89 + (n*(M+1)+u | 88)]
        augp = np.zeros((128, 4, AUG), f)
        wau_c = Wau[h0:h0 + HPC]                       # [8, 512, 10]
        grid = np.empty((128, 4, HPC, M + 1), f)
        grid[:, :, :, 0:M] = wau_c.reshape(HPC, 4, 128, M).transpose(2, 1, 0, 3)
        grid[:, :, :, M] = Wvu[h0:h0 + HPC].reshape(HPC, 4, 128).transpose(2, 1, 0)
        augp[:, :, 0:HPC * (M + 1)] = grid.reshape(128, 4, HPC * (M + 1))
        augp[:, :, AUG - 1] = Wvc.reshape(4, 128).T
        augbp = np.zeros((1, AUG), f)
        bgrid = np.empty((HPC, M + 1), f)
        bgrid[:, 0:M] = np.asarray(bau, f)[h0:h0 + HPC]
        bgrid[:, M] = np.asarray(bvu, f)[h0:h0 + HPC]
        augbp[0, 0:HPC * (M + 1)] = bgrid.reshape(-1)
        augbp[0, AUG - 1] = np.asarray(bvc, f).reshape(1)[0]
        m = {
            "xt": xtp.astype(BF),
            "w1g0": np.ascontiguousarray(
                w1r[0:13].transpose(1, 0, 2)).reshape(128, 13 * H).astype(BF),
            "w1g1": np.ascontiguousarray(
                w1r[13:26].transpose(1, 0, 2)).reshape(128, 13 * H).astype(BF),
            "b1": b1r, "w2": w2p, "b2": b2r,
            "wac": wacp.astype(BF),
            "bac": np.asarray(bac, f)[r0:r0 + RPC].reshape(1, RPC).astype(BF),
            "wru": wrup.astype(BF),
            "bru": np.ascontiguousarray(
                np.asarray(bru, f)[:, r0:r0 + RPC]).astype(BF),
            "aug": augp.reshape(128, 4 * AUG).astype(BF),
            "augb": augbp.astype(BF),
            "sel4": sel4p,
        }
        in_maps.append(m)
    return in_maps


def assemble(results):
    q = np.empty((B, 2 * R + NH * M), np.float32)
    for c in range(NC):
        r0 = c * RPC
        a0 = c * HPC * M
        q[:, r0:r0 + RPC] = results[c]["out_cache"]
        q[:, R + r0:R + r0 + RPC] = results[c]["out_rec"]
        q[:, 2 * R + a0:2 * R + a0 + HPC * M] = results[c]["out_assoc"]
    return q


def run(in_maps, **kw):
    nc = _get_program()
    return bass_utils.run_bass_kernel_spmd(nc, in_maps, core_ids=list(range(NC)), **kw)


def kernel(**inputs):
    in_maps = make_in_maps(**{k: np.asarray(v) for k, v in inputs.items()})
    res = run(in_maps)
    return assemble(res.results)


# revision 44
# speedup vs baseline: 1.2722x; 1.2722x over previous
"""Trainium2 Bass kernel for nn_MultiHeadDuelingDQN (8-core SPMD), bf16 edition.

Model (B=256, STATE=26240, H=512, R=4000, N=64 heads, M=10):
    h  = relu(relu(x@W1+b1)@W2+b2)
    q_cache = h@Wvc+bvc + (h@Wac+bac) - mean_R(h@Wac+bac)
    q_assoc = per-head dueling over M (local means)
    q_rec   = S - mean_R(S),  S = h @ (sum_n Wru[n]) + sum_n bru[n]
              [exact rewrite: rec_global has zero row-mean, so the
              reference's second mean subtraction folds away]

Sharding (8 cores), all streamed tensors staged as bf16 on host:
  - fc1: contraction (STATE) split 8 ways. x and W1 are pre-transposed /
    supertiled on the host so fc1 is pure matmul (no on-device transposes).
    Partial h1 [256,512] summed across cores via ReduceScatter (bf16, CCE
    adds in the DMA path); each core computes fc2 on its 32 batch rows and
    AllGathers h2 (bf16).
  - rec/cache: R split 8 ways (500 cols/core). sum_n Wru[n] becomes DVE
    grouped reduces over big [128, 16000] bf16 chunks (heads innermost),
    then one small matmul h @ W_sum per batch tile. Row-means over the full
    R use a tiny [128,4] f32 AllGather + local reduce.
  - assoc heads: 8 heads/core, fully local; augmented matmul
    [Wau | Wvu | Wvc] -> [adv_assoc | val_n | value_c] in one pass.
Engine split: PE does fc/head matmuls + the few h transposes; DVE only the
Wru stream reduction; ACT does PSUM->SBUF copies/relus/casts, row-sums
(accum_out), mean subtraction (Identity+bias) and load-DMA dispatch; Sync
dispatches the big input streams (xT, W1, Wru); GpSimd runs collectives,
output stores and small SBUF elementwise ops.

kernel(**inputs) takes full unsharded f32 inputs, returns full [256, 8640] f32.
"""
import numpy as np
import ml_dtypes

import concourse.bass as bass
import concourse.mybir as mybir
import concourse.tile as tile
from concourse import bacc
from concourse import bass_utils
from concourse.bass import ts
from concourse.masks import make_identity

NC = 8
B, H, STATE, R, NH, M = 256, 512, 26240, 4000, 64, 10
KPC_RAW = STATE // NC          # 3280
KCH = 26                       # k-chunks of 128 per core (padded)
KPC = KCH * 128                # 3328
RPC = R // NC                  # 500
HPC = NH // NC                 # 8 heads per core
AUG = HPC * (M + 1) + 1        # 89 = [8x(10 adv + 1 val)] + value_c
BPC = B // NC                  # 32 batch rows per core for fc2
F32 = mybir.dt.float32
BF16 = mybir.dt.bfloat16
BF = ml_dtypes.bfloat16
RELU = mybir.ActivationFunctionType.Relu
COPY = mybir.ActivationFunctionType.Copy
IDENT = mybir.ActivationFunctionType.Identity
ADD = mybir.AluOpType.add

# wru stream chunking: (kc, r-offset, r-count) pieces; last chunk halved so
# the final DVE tree (which gates S) is short
RN = 125
WRU_CHUNKS = ([(kc, rq * RN, RN) for kc in range(4) for rq in range(4)][:-1]
              + [(3, 375, 62), (3, 437, 63)])


def build_program(wru_bufs=4):
    nc = bacc.Bacc("TRN2", target_bir_lowering=False, debug=False, num_devices=NC)

    # ---- per-core I/O (host pre-packs layouts; see make_in_maps) ----
    xt = nc.dram_tensor("xt", [128, KCH * 256], BF16, kind="ExternalInput").ap()
    w1g = [nc.dram_tensor(f"w1g{g}", [128, 13 * 512], BF16, kind="ExternalInput").ap()
           for g in range(2)]
    b1 = nc.dram_tensor("b1", [1, H], BF16, kind="ExternalInput").ap()
    w2 = nc.dram_tensor("w2", [128, 4 * 512], BF16, kind="ExternalInput").ap()
    b2 = nc.dram_tensor("b2", [1, H], BF16, kind="ExternalInput").ap()
    wac = nc.dram_tensor("wac", [128, 4 * RPC], BF16, kind="ExternalInput").ap()
    bac = nc.dram_tensor("bac", [1, RPC], BF16, kind="ExternalInput").ap()
    # [kc, p, r*64 + n] = Wru[n, kc*128+p, r0+r]  (heads innermost)
    wru = nc.dram_tensor("wru", [4, 128, RPC * NH], BF16, kind="ExternalInput").ap()
    bru = nc.dram_tensor("bru", [NH, RPC], BF16, kind="ExternalInput").ap()
    aug = nc.dram_tensor("aug", [128, 4 * AUG], BF16, kind="ExternalInput").ap()
    augb = nc.dram_tensor("augb", [1, AUG], BF16, kind="ExternalInput").ap()
    # selection matrix: sel4[g*32+p, p] = 1 — sums 4 stacked partials via PE
    sel4 = nc.dram_tensor("sel4", [128, BPC], BF16, kind="ExternalInput").ap()

    out_cache = nc.dram_tensor("out_cache", [B, RPC], F32, kind="ExternalOutput").ap()
    out_rec = nc.dram_tensor("out_rec", [B, RPC], F32, kind="ExternalOutput").ap()
    out_assoc = nc.dram_tensor("out_assoc", [B, HPC * M], F32, kind="ExternalOutput").ap()

    with tile.TileContext(nc) as tc:
        with (
            tc.tile_pool(name="cst", bufs=1) as cst,
            tc.tile_pool(name="sb", bufs=1) as sb,
            tc.tile_pool(name="wrup", bufs=wru_bufs) as wrup,
            tc.tile_pool(name="redp", bufs=2) as redp,
            tc.tile_pool(name="ps", bufs=2, space="PSUM") as ps,
            tc.tile_pool(name="psfc", bufs=2, space="PSUM") as psfc,
            tc.tile_pool(name="dram", bufs=1, space="DRAM") as dram,
        ):
            ident = cst.tile([128, 128], BF16, tag="ident")
            make_identity(nc, ident)
            ones8 = cst.tile([1, 128], BF16, tag="ones8")
            nc.vector.memset(ones8, 1.0 / NC)
            ones1 = cst.tile([1, 128], BF16, tag="ones1")
            nc.vector.memset(ones1, 1.0)
            ones64 = cst.tile([64, 128], BF16, tag="ones64")
            nc.vector.memset(ones64, 1.0)

            # ---------- Sync HWDGE ring: trunk inputs first, then wru stream --
            xt_sb = sb.tile([128, KCH * 256], BF16, tag="xt_sb")
            nc.sync.dma_start(xt_sb, xt)
            w1_sb = []
            for g in range(2):
                t = sb.tile([128, 13 * 512], BF16, tag=f"w1_{g}", name=f"w1_{g}")
                nc.sync.dma_start(t, w1g[g])
                w1_sb.append(t)

            # ---------- small loads on the Scalar HWDGE ring ----------
            b1row = cst.tile([1, H], BF16, tag="b1row")
            nc.scalar.dma_start(b1row, b1)
            w2_sb = sb.tile([128, 4 * 512], BF16, tag="w2_sb")
            nc.scalar.dma_start(w2_sb, w2)
            b2row = cst.tile([1, H], BF16, tag="b2row")
            nc.scalar.dma_start(b2row, b2)
            wac_sb = sb.tile([128, 4 * RPC], BF16, tag="wac_sb")
            nc.scalar.dma_start(wac_sb, wac)
            bac_row = cst.tile([1, RPC], BF16, tag="bac_row")
            nc.scalar.dma_start(bac_row, bac)
            aug_sb = cst.tile([128, 4 * AUG], BF16, tag="aug_sb")
            nc.scalar.dma_start(aug_sb, aug)
            augb_row = cst.tile([1, AUG], BF16, tag="augb_row")
            nc.scalar.dma_start(augb_row, augb)
            bru_sb = sb.tile([64, RPC], BF16, tag="bru_sb")
            nc.scalar.dma_start(bru_sb, bru)

            # wru stream + DVE head-sum as a binary TT tree (bf16 2x packed
            # mode; tensor_reduce only runs 1x on TRN2)
            acc_bf = [sb.tile([128, RPC], BF16, tag=f"accb{k}", name=f"accb{k}")
                      for k in range(4)]
            # ---------- Phase A: fc1 (pure matmul; x/W1 pre-transposed) ----
            h1_ps = [psfc.tile([128, H], F32, tag="fc", name=f"h1_ps{bt}")
                     for bt in range(2)]
            for bt in range(2):  # fold b1/8 first, opens the accumulation group
                nc.tensor.matmul(h1_ps[bt], ones8, b1row, start=True, stop=False)
            for kc in range(KCH):
                w1t = w1_sb[kc // 13]
                j = kc % 13
                for bt in range(2):
                    nc.tensor.matmul(h1_ps[bt],
                                     xt_sb[:, kc * 256 + bt * 128:kc * 256 + (bt + 1) * 128],
                                     w1t[:, ts(j, 512)],
                                     start=False, stop=(kc == KCH - 1))
            h1c = []
            rs_in_early = dram.tile([B, H], BF16, tag="rs_in")
            for bt in range(2):
                t = sb.tile([128, H], BF16, tag=f"h1c{bt}", name=f"h1c{bt}")
                nc.scalar.copy(t, h1_ps[bt])
                nc.gpsimd.dma_start(rs_in_early[ts(bt, 128), :], t)
                h1c.append(t)

            with nc.allow_low_precision(reason="DVE rounds each tree level to bf16"):
                for ci, (kc, rb, rn) in enumerate(WRU_CHUNKS):
                    wt = wrup.tile([128, rn * NH], BF16, tag="wru", name=f"wru_t{ci}")
                    nc.sync.dma_start(wt, wru[kc, :, rb * NH:(rb + rn) * NH])
                    sA = redp.tile([128, rn * 32], BF16, tag="sA", name=f"sA{ci}")
                    sB = redp.tile([128, rn * 16], BF16, tag="sB", name=f"sB{ci}")
                    cur = wt
                    dsts = [sA[:, 0:rn * 32], sB[:, 0:rn * 16], sA[:, 0:rn * 8],
                            sB[:, 0:rn * 4], sA[:, 0:rn * 2],
                            acc_bf[kc][:, rb:rb + rn]]
                    for lvl, g in enumerate([64, 32, 16, 8, 4, 2]):
                        h = g // 2
                        in0 = bass.AP(cur.tensor, cur.offset,
                                      [cur.ap[0], [g, rn], [1, h]])
                        in1 = bass.AP(cur.tensor, cur.offset + h,
                                      [cur.ap[0], [g, rn], [1, h]])
                        nc.vector.tensor_tensor(out=dsts[lvl], in0=in0, in1=in1,
                                                op=ADD)
                        cur = dsts[lvl]

            # AllToAll h1 (bf16): one-hop mesh exchange; rank c receives the 8
            # partials of its 32 batch rows, then sums them with a PE sel-matmul
            rs_in = rs_in_early
            rs_out = dram.tile([B, H], BF16, tag="rs_out")
            nc.gpsimd.collective_compute(
                "AllToAll", mybir.AluOpType.bypass,
                replica_groups=[list(range(NC))],
                ins=[rs_in.opt()], outs=[rs_out.opt()],
            )
            # contiguous readback of the 8 stacked partials, then sum them
            # across partitions with two PE matmuls against sel4
            sel_sb = cst.tile([128, BPC], BF16, tag="sel_sb")
            nc.scalar.dma_start(sel_sb, sel4)
            h1sum_ps = psfc.tile([BPC, H], F32, tag="fc", name="h1sum_ps")
            for bt in range(2):
                h1pt = sb.tile([128, H], BF16, tag=f"h1pt{bt}", name=f"h1pt{bt}")
                nc.gpsimd.dma_start(h1pt, rs_out[ts(bt, 128), :])
                nc.tensor.matmul(h1sum_ps, sel_sb, h1pt,
                                 start=(bt == 0), stop=(bt == 1))
            h1s = sb.tile([BPC, H], BF16, tag="h1s")
            nc.scalar.activation(h1s, h1sum_ps, RELU)
            h1cT = []
            for kc in range(4):
                pt = ps.tile([128, BPC], BF16, tag="small", bufs=2, name=f"pth{kc}")
                nc.tensor.transpose(pt, h1s[:, ts(kc, 128)], ident[0:BPC, 0:BPC])
                t = sb.tile([128, BPC], BF16, tag=f"h1cT{kc}", name=f"h1cT{kc}")
                nc.scalar.copy(t, pt)
                h1cT.append(t)
            h2_ps = psfc.tile([BPC, H], F32, tag="fc", name="h2_ps")
            nc.tensor.matmul(h2_ps, ones1[:, 0:BPC], b2row, start=True, stop=False)
            for kc in range(4):
                nc.tensor.matmul(h2_ps, h1cT[kc], w2_sb[:, ts(kc, 512)],
                                 start=False, stop=(kc == 3))
            h2s = sb.tile([BPC, H], BF16, tag="h2s")
            nc.scalar.activation(h2s, h2_ps, RELU)
            ag_in = dram.tile([BPC, H], BF16, tag="ag_in")
            ag_out = dram.tile([B, H], BF16, tag="ag_out")
            nc.gpsimd.dma_start(ag_in, h2s)
            nc.gpsimd.collective_compute(
                "AllGather", mybir.AluOpType.bypass,
                replica_groups=[list(range(NC))],
                ins=[ag_in.opt()], outs=[ag_out.opt()],
            )
            # h2 [256, 512] -> hT chunks [128(h), 256(b)]
            hT = [sb.tile([128, B], BF16, tag=f"hT{kc}", name=f"hT{kc}")
                  for kc in range(4)]
            for bt in range(2):
                h2g = sb.tile([128, H], BF16, tag=f"h2g{bt}", name=f"h2g{bt}")
                nc.gpsimd.dma_start(h2g, ag_out[ts(bt, 128), :])
                for kc in range(4):
                    pt = ps.tile([128, 128], BF16, tag="small", bufs=2,
                                 name=f"ptg{bt}_{kc}")
                    nc.tensor.transpose(pt, h2g[:, ts(kc, 128)], ident)
                    nc.scalar.copy(hT[kc][:, ts(bt, 128)], pt)

            # ---------- Phase B: assoc heads (augmented [adv|val|value_c]) ----
            # (matmul+copy only; the dueling finalize is emitted after the
            # ar2-critical psC/psS copies so it stays off that ACT path)
            value_sb, psA_sbs = [], []
            for bt in range(2):
                psA = ps.tile([128, AUG], F32, tag="wide", bufs=3, name=f"psA{bt}")
                nc.tensor.matmul(psA, ones1, augb_row, start=True, stop=False)
                for kc in range(4):
                    nc.tensor.matmul(psA, hT[kc][:, ts(bt, 128)],
                                     aug_sb[:, kc * AUG:(kc + 1) * AUG],
                                     start=False, stop=(kc == 3))
                psA_sb = sb.tile([128, AUG], F32, tag=f"psAsb{bt}", name=f"psAsb{bt}")
                nc.scalar.copy(psA_sb, psA)
                psA_sbs.append(psA_sb)
                value_sb.append(psA_sb[:, AUG - 1:AUG])

            # ---------- Phase C: cache head (R-slice) ----------
            ar2_in = sb.tile([128, 4], F32, tag="ar2_in")
            adv_c_sb = []
            for bt in range(2):
                psC = ps.tile([128, RPC], F32, tag="wide", bufs=3, name=f"psC{bt}")
                nc.tensor.matmul(psC, ones1, bac_row, start=True, stop=False)
                for kc in range(4):
                    nc.tensor.matmul(psC, hT[kc][:, ts(bt, 128)],
                                     wac_sb[:, kc * RPC:(kc + 1) * RPC],
                                     start=False, stop=(kc == 3))
                t = sb.tile([128, RPC], F32, tag=f"advc{bt}", name=f"advc{bt}")
                # copy + row-sum in one ACT pass (accum_out)
                nc.scalar.activation(t, psC, COPY,
                                     accum_out=ar2_in[:, bt:bt + 1])
                adv_c_sb.append(t)

            # ---------- Phase D: S = hT.T @ W_sum (+ sum_n bru fold) ----------
            s_sb = []
            for bt in range(2):
                t = ps.tile([128, RPC], F32, tag="wide", bufs=3, name=f"psS{bt}")
                nc.tensor.matmul(t, ones64, bru_sb, start=True, stop=False)
                for kc in range(4):
                    nc.tensor.matmul(t, hT[kc][:, ts(bt, 128)], acc_bf[kc],
                                     start=False, stop=(kc == 3))
                st = sb.tile([128, RPC], F32, tag=f"ssb{bt}", name=f"ssb{bt}")
                nc.scalar.activation(st, t, COPY,
                                     accum_out=ar2_in[:, 2 + bt:3 + bt])
                s_sb.append(st)

            # ---------- assoc dueling finalize (off the ar2 ACT path) ----------
            for bt in range(2):
                psA_sb = psA_sbs[bt]
                advs = psA_sb[:, 0:HPC * (M + 1)].rearrange("p (n u) -> p n u", u=M + 1)
                negm = sb.tile([128, HPC], F32, tag=f"negmA{bt}", name=f"negmA{bt}")
                junkA = sb.tile([128, M], F32, tag=f"junkA{bt}", name=f"junkA{bt}")
                for n in range(HPC):
                    nc.scalar.activation(junkA, advs[:, n, 0:M], COPY,
                                         scale=-1.0 / M,
                                         accum_out=negm[:, n:n + 1])
                tmp = sb.tile([128, HPC], F32, tag=f"tmpA{bt}", name=f"tmpA{bt}")
                nc.gpsimd.tensor_add(tmp, advs[:, :, M], negm)
                q = sb.tile([128, HPC * M], F32, tag=f"qA{bt}", name=f"qA{bt}")
                nc.gpsimd.tensor_tensor(
                    out=q.rearrange("p (n m) -> p n m", m=M),
                    in0=advs[:, :, 0:M],
                    in1=tmp.broadcast_to([128, HPC, M]),
                    op=ADD)
                nc.scalar.dma_start(out_assoc[ts(bt, 128), :], q)

            # ---------- Phase E: tiny AllGather of row-sums, finalize ----------
            ar2_din = dram.tile([128, 4], F32, tag="ar2_din")
            ar2_dout = dram.tile([NC * 128, 4], F32, tag="ar2_dout")
            nc.sync.dma_start(ar2_din, ar2_in)
            nc.gpsimd.collective_compute(
                "AllGather", mybir.AluOpType.bypass,
                replica_groups=[list(range(NC))],
                ins=[ar2_din.opt()], outs=[ar2_dout.opt()],
            )
            # one strided readback [128, (g,c)] then a single X-reduce over g
            rall = sb.tile([128, NC * 4], F32, tag="rall")
            nc.sync.dma_start(rall, ar2_dout.rearrange("(g p) c -> p g c", p=128))
            rview = bass.AP(rall.tensor, rall.offset,
                            [rall.ap[0], [1, 4], [4, NC]])
            ar2_sb = sb.tile([128, 4], F32, tag="ar2_sb")
            nc.vector.tensor_reduce(ar2_sb, rview, axis=mybir.AxisListType.X, op=ADD)
            negmeans = sb.tile([128, 4], F32, tag="negmeans")
            nc.scalar.activation(negmeans, ar2_sb, COPY, scale=-1.0 / R)

            for bt in range(2):
                vm = sb.tile([128, 1], F32, tag=f"vm{bt}", name=f"vm{bt}")
                nc.gpsimd.tensor_add(vm, value_sb[bt], negmeans[:, bt:bt + 1])
                qc = sb.tile([128, RPC], F32, tag=f"qc{bt}", name=f"qc{bt}")
                nc.scalar.activation(qc, adv_c_sb[bt], IDENT, bias=vm, scale=1.0)
                nc.scalar.dma_start(out_cache[ts(bt, 128), :], qc)

                qr = sb.tile([128, RPC], F32, tag=f"qr{bt}", name=f"qr{bt}")
                nc.scalar.activation(qr, s_sb[bt], IDENT,
                                     bias=negmeans[:, 2 + bt:3 + bt], scale=1.0)
                nc.scalar.dma_start(out_rec[ts(bt, 128), :], qr)

    nc.compile()
    return nc


_CACHED = None


def _get_program():
    global _CACHED
    if _CACHED is None:
        _CACHED = build_program()
    return _CACHED


def make_in_maps(x, W1, b1, W2, b2, Wvc, bvc, Wac, bac, Wvu, bvu, Wau, bau, Wru, bru):
    f = np.float32
    x = np.asarray(x, f)
    W1 = np.asarray(W1, f)
    W2 = np.asarray(W2, f)
    Wac = np.asarray(Wac, f)
    Wru = np.asarray(Wru, f)
    Wau = np.asarray(Wau, f)
    Wvu = np.asarray(Wvu, f)
    Wvc = np.asarray(Wvc, f).reshape(H)
    in_maps = []
    sel4p = np.zeros((128, BPC), np.float32)
    sel4p[np.arange(128), np.arange(128) % BPC] = 1.0
    sel4p = sel4p.astype(BF)
    w2p = np.ascontiguousarray(
        W2.reshape(4, 128, H).transpose(1, 0, 2)).reshape(128, 4 * H).astype(BF)
    b1r = np.asarray(b1, f).reshape(1, H).astype(BF)
    b2r = np.asarray(b2, f).reshape(1, H).astype(BF)
    for c in range(NC):
        k0 = c * KPC_RAW
        r0 = c * RPC
        h0 = c * HPC
        # xT supertile: [p, kc*256 + b] = x[b, k0 + kc*128 + p]
        xs = np.zeros((KPC, B), f)
        xs[:KPC_RAW] = x[:, k0:k0 + KPC_RAW].T
        xtp = xs.reshape(KCH, 128, B).transpose(1, 0, 2).reshape(128, KCH * B)
        # W1 supertiles: [p, j*512 + h] = W1[k0 + (g*13+j)*128 + p, h]
        w1s = np.zeros((KPC, H), f)
        w1s[:KPC_RAW] = W1[k0:k0 + KPC_RAW]
        w1r = w1s.reshape(KCH, 128, H)
        # wru chunks: [kc, p, r*64 + n] = Wru[n, kc*128+p, r0+r]
        ws = Wru[:, :, r0:r0 + RPC]                    # [64, 512, 500]
        wrup = np.ascontiguousarray(
            ws.reshape(NH, 4, 128, RPC).transpose(1, 2, 3, 0)).reshape(
            4, 128, RPC * NH)
        # wac supertile: [p, kc*500 + r] = Wac[kc*128+p, r0+r]
        wacp = np.ascontiguousarray(
            Wac[:, r0:r0 + RPC].reshape(4, 128, RPC).transpose(1, 0, 2)).reshape(
            128, 4 * RPC)
        # augmented assoc weights: [p, kc*89 + (n*(M+1)+u | 88)]
        augp = np.zeros((128, 4, AUG), f)
        wau_c = Wau[h0:h0 + HPC]                       # [8, 512, 10]
        grid = np.empty((128, 4, HPC, M + 1), f)
        grid[:, :, :, 0:M] = wau_c.reshape(HPC, 4, 128, M).transpose(2, 1, 0, 3)
        grid[:, :, :, M] = Wvu[h0:h0 + HPC].reshape(HPC, 4, 128).transpose(2, 1, 0)
        augp[:, :, 0:HPC * (M + 1)] = grid.reshape(128, 4, HPC * (M + 1))
        augp[:, :, AUG - 1] = Wvc.reshape(4, 128).T
        augbp = np.zeros((1, AUG), f)
        bgrid = np.empty((HPC, M + 1), f)
        bgrid[:, 0:M] = np.asarray(bau, f)[h0:h0 + HPC]
        bgrid[:, M] = np.asarray(bvu, f)[h0:h0 + HPC]
        augbp[0, 0:HPC * (M + 1)] = bgrid.reshape(-1)
        augbp[0, AUG - 1] = np.asarray(bvc, f).reshape(1)[0]
        m = {
            "xt": xtp.astype(BF),
            "w1g0": np.ascontiguousarray(
                w1r[0:13].transpose(1, 0, 2)).reshape(128, 13 * H).astype(BF),
            "w1g1": np.ascontiguousarray(
                w1r[13:26].transpose(1, 0, 2)).reshape(128, 13 * H).astype(BF),
            "b1": b1r, "w2": w2p, "b2": b2r,
            "wac": wacp.astype(BF),
            "bac": np.asarray(bac, f)[r0:r0 + RPC].reshape(1, RPC).astype(BF),
            "wru": wrup.astype(BF),
            "bru": np.ascontiguousarray(
                np.asarray(bru, f)[:, r0:r0 + RPC]).astype(BF),
            "aug": augp.reshape(128, 4 * AUG).astype(BF),
            "augb": augbp.astype(BF),
            "sel4": sel4p,
        }
        in_maps.append(m)
    return in_maps


def assemble(results):
    q = np.empty((B, 2 * R + NH * M), np.float32)
    for c in range(NC):
        r0 = c * RPC
        a0 = c * HPC * M
        q[:, r0:r0 + RPC] = results[c]["out_cache"]
        q[:, R + r0:R + r0 + RPC] = results[c]["out_rec"]
        q[:, 2 * R + a0:2 * R + a0 + HPC * M] = results[c]["out_assoc"]
    return q


def run(in_maps, **kw):
    nc = _get_program()
    return bass_utils.run_bass_kernel_spmd(nc, in_maps, core_ids=list(range(NC)), **kw)


def kernel(**inputs):
    in_maps = make_in_maps(**{k: np.asarray(v) for k, v in inputs.items()})
    res = run(in_maps)
    return assemble(res.results)
